# revision 2
# baseline (speedup 1.0000x reference)
"""Bahdanau attention forward on 8 Trainium2 NeuronCores.

reference (per batch row b):
    rq = query * W1[0]                      # [E]
    s  = tanh(rq[None, :] * (value[b] * W2))  # [W, E]
    scores = sum(s * W3, -1)                # [W]
    a = softmax(scores)                     # [W]
    out[b] = a @ value[b]                   # [E]

Sharding: pure data-parallel over batch (65536 rows -> 8 cores x 8192).
W1/W2 are folded into one fp16 [128, W*E] replicated constant on the host
(W12[w,e] = W1[e]*W2[w,e]); likewise W3 is replicated to [128, W*E].

Per-core device kernel (natural layout, batch on partitions):
  tile = 128 batch rows x (W*E = 2560) free elems, 64 tiles.
  1. v16  <- cast-DMA (fp32->fp16) of value tile          [128, 2560]
  2. q16  <- cast-DMA of query tile                       [128, 128]
  3. h    = v16 * q16 broadcast along W (DVE TT, fp16 2x) [128, 2560]
  4. h   *= W12rep (DVE TT, fp16 2x)
  5. s    = tanh(h) (ACT, in-place)
  6. scores[:, w] = sum_e s[:,w,:]*W3rep[:,w,:]  -- 20 fused
     scalar_tensor_tensor ops with fp32 accum_out (DVE)
  7. softmax over W in fp32 (max-sub, exp on ACT, recip on DVE),
     a16 = fp16 normalized weights                        [128, 20]
  8. ctx = sum_w a16[:,w] * v16[:,w,:] -- tensor_scalar + 19 fused
     multiply-accumulate scalar_tensor_tensor ops (DVE, fp16)
  9. cast-DMA ctx -> fp32 DRAM out
"""

import numpy as np

B, W, E = 65536, 20, 128
N_CORES = 8
B_CORE = B // N_CORES

_CACHE = {}


def _build(b_core: int):
    """Build + compile the per-core Bass program. Returns (nc, names)."""
    import sys

    if "/opt/trn_rl_repo" not in sys.path:
        sys.path.insert(0, "/opt/trn_rl_repo")
    import concourse.bacc as bacc
    import concourse.mybir as mybir
    import concourse.tile as tile

    f16 = mybir.dt.float16
    f32 = mybir.dt.float32
    WE = W * E
    n_tiles = b_core // 128
    assert b_core % 128 == 0

    nc = bacc.Bacc(
        "TRN2",
        target_bir_lowering=False,
        debug=False,
        enable_asserts=False,
        num_devices=N_CORES,
    )

    value_d = nc.dram_tensor("value", [b_core, W, E], f32, kind="ExternalInput").ap()
    query_d = nc.dram_tensor("query", [b_core, E], f32, kind="ExternalInput").ap()
    w12_d = nc.dram_tensor("w12rep", [128, WE], f16, kind="ExternalInput").ap()
    w3_d = nc.dram_tensor("w3rep", [128, WE], f16, kind="ExternalInput").ap()
    ctx_d = nc.dram_tensor("ctx", [b_core, E], f32, kind="ExternalOutput").ap()

    value_f = value_d.rearrange("b w e -> b (w e)")

    mult = mybir.AluOpType.mult
    add = mybir.AluOpType.add
    bypass = mybir.AluOpType.bypass
    AXX = mybir.AxisListType.X
    Tanh = mybir.ActivationFunctionType.Tanh
    Exp = mybir.ActivationFunctionType.Exp

    with tile.TileContext(nc) as tc:
        with (
            tc.tile_pool(name="consts", bufs=1) as cpool,
            tc.tile_pool(name="vbuf", bufs=3) as vpool,
            tc.tile_pool(name="hbuf", bufs=2) as hpool,
            tc.tile_pool(name="qbuf", bufs=2) as qpool,
            tc.tile_pool(name="small", bufs=2) as spool,
            tc.tile_pool(name="ctxbuf", bufs=2) as opool,
        ):
            w12 = cpool.tile([128, WE], f16, tag="w12")
            nc.sync.dma_start(w12[:], w12_d)
            w3 = cpool.tile([128, WE], f16, tag="w3")
            nc.sync.dma_start(w3[:], w3_d)
            w12_3 = w12[:].rearrange("p (w e) -> p w e", w=W)
            w3_3 = w3[:].rearrange("p (w e) -> p w e", w=W)

            for i in range(n_tiles):
                rows = slice(i * 128, (i + 1) * 128)

                v16 = vpool.tile([128, WE], f16)
                nc.gpsimd.dma_start(v16[:], value_f[rows, :])
                v3 = v16[:].rearrange("p (w e) -> p w e", w=W)

                q16 = qpool.tile([128, E], f16)
                nc.gpsimd.dma_start(q16[:], query_d[rows, :])

                # h = v * q (q broadcast along W), then h *= W12rep
                h = hpool.tile([128, WE], f16)
                h3 = h[:].rearrange("p (w e) -> p w e", w=W)
                qb = q16[:].unsqueeze(1).broadcast_to([128, W, E])
                nc.vector.tensor_tensor(h3, v3, qb, mult)
                nc.vector.tensor_tensor(h[:], h[:], w12[:], mult)

                # s = tanh(h) in place
                nc.scalar.activation(h[:], h[:], Tanh)

                # scores[:, w] = sum_e s[:, w, :] * W3rep[:, w, :]
                scores = spool.tile([128, W], f32, tag="scores")
                for w in range(W):
                    nc.vector.scalar_tensor_tensor(
                        out=h3[:, w, :],
                        in0=h3[:, w, :],
                        scalar=1.0,
                        in1=w3_3[:, w, :],
                        op0=bypass,
                        op1=mult,
                        accum_out=scores[:, w : w + 1],
                    )

                # softmax over W (fp32), folded normalization
                m = spool.tile([128, 1], f32, tag="m")
                nc.vector.tensor_reduce(m[:], scores[:], AXX, mybir.AluOpType.max)
                negm = spool.tile([128, 1], f32, tag="negm")
                nc.scalar.mul(negm[:], m[:], -1.0)
                e32 = spool.tile([128, W], f32, tag="e32")
                nc.scalar.activation(e32[:], scores[:], Exp, bias=negm[:])
                ssum = spool.tile([128, 1], f32, tag="ssum")
                nc.vector.tensor_reduce(ssum[:], e32[:], AXX, add)
                rec = spool.tile([128, 1], f32, tag="rec")
                nc.vector.reciprocal(rec[:], ssum[:])
                a32 = spool.tile([128, W], f32, tag="a32")
                nc.vector.tensor_scalar(a32[:], e32[:], rec[:], None, mult)

                # ctx = sum_w a[:, w] * v[:, w, :]
                ctx16 = opool.tile([128, E], f16)
                nc.vector.tensor_scalar(ctx16[:], v3[:, 0, :], a32[:, 0:1], None, mult)
                for w in range(1, W):
                    nc.vector.scalar_tensor_tensor(
                        out=ctx16[:],
                        in0=v3[:, w, :],
                        scalar=a32[:, w : w + 1],
                        in1=ctx16[:],
                        op0=mult,
                        op1=add,
                    )

                nc.gpsimd.dma_start(ctx_d[rows, :], ctx16[:])

    nc.compile()
    return nc


def _get_nc(b_core: int):
    if b_core not in _CACHE:
        _CACHE[b_core] = _build(b_core)
    return _CACHE[b_core]


def _host_weights(W1, W2, W3):
    w12 = (W1.astype(np.float32)[0][None, :] * W2.astype(np.float32)).reshape(-1)
    w12rep = np.broadcast_to(w12, (128, W * E)).astype(np.float16)
    w3rep = np.broadcast_to(
        W3.astype(np.float32).reshape(-1), (128, W * E)
    ).astype(np.float16)
    return np.ascontiguousarray(w12rep), np.ascontiguousarray(w3rep)


def kernel(query, value, W1, W2, W3):
    import sys

    if "/opt/trn_rl_repo" not in sys.path:
        sys.path.insert(0, "/opt/trn_rl_repo")
    from concourse.bass_utils import run_bass_kernel_spmd

    query = np.asarray(query, dtype=np.float32)
    value = np.asarray(value, dtype=np.float32)
    w12rep, w3rep = _host_weights(np.asarray(W1), np.asarray(W2), np.asarray(W3))

    nc = _get_nc(B_CORE)
    in_maps = []
    for c in range(N_CORES):
        rows = slice(c * B_CORE, (c + 1) * B_CORE)
        in_maps.append(
            {
                "value": np.ascontiguousarray(value[rows]),
                "query": np.ascontiguousarray(query[rows]),
                "w12rep": w12rep,
                "w3rep": w3rep,
            }
        )

    res = run_bass_kernel_spmd(nc, in_maps, list(range(N_CORES)))
    out = np.concatenate([res.results[c]["ctx"] for c in range(N_CORES)], axis=0)
    return out.astype(np.float32)


# revision 4
# speedup vs baseline: 18.3454x; 18.3454x over previous
"""Bahdanau attention forward on 8 Trainium2 NeuronCores.

reference (per batch row b):
    rq = query * W1[0]                      # [E]
    s  = tanh(rq[None, :] * (value[b] * W2))  # [W, E]
    scores = sum(s * W3, -1)                # [W]
    a = softmax(scores)                     # [W]
    out[b] = a @ value[b]                   # [E]

Sharding: pure data-parallel over batch (65536 rows -> 8 cores x 8192).
W1/W2 are folded into one fp16 [128, W*E] replicated constant on the host
(W12[w,e] = W1[e]*W2[w,e]); likewise W3 is replicated to [128, W*E].

Per-core device kernel (natural layout, batch on partitions):
  tile = 128 batch rows x (W*E = 2560) free elems, 64 tiles.
  1. v16  <- cast-DMA (fp32->fp16) of value tile          [128, 2560]
  2. q16  <- cast-DMA of query tile                       [128, 128]
  3. h    = v16 * q16 broadcast along W (DVE TT, fp16 2x) [128, 2560]
  4. h   *= W12rep (DVE TT, fp16 2x)
  5. s    = tanh(h) (ACT, in-place)
  6. scores[:, w] = sum_e s[:,w,:]*W3rep[:,w,:]  -- 20 fused
     scalar_tensor_tensor ops with fp32 accum_out (DVE)
  7. softmax over W in fp32 (max-sub, exp on ACT, recip on DVE),
     a16 = fp16 normalized weights                        [128, 20]
  8. ctx = sum_w a16[:,w] * v16[:,w,:] -- tensor_scalar + 19 fused
     multiply-accumulate scalar_tensor_tensor ops (DVE, fp16)
  9. cast-DMA ctx -> fp32 DRAM out
"""

import numpy as np

B, W, E = 65536, 20, 128
N_CORES = 8
B_CORE = B // N_CORES

_CACHE = {}


def _build(b_core: int, reps: int = 1):
    """Build + compile the per-core Bass program. Returns (nc, names)."""
    import sys

    if "/opt/trn_rl_repo" not in sys.path:
        sys.path.insert(0, "/opt/trn_rl_repo")
    import concourse.bacc as bacc
    import concourse.mybir as mybir
    import concourse.tile as tile

    f16 = mybir.dt.float16
    f32 = mybir.dt.float32
    WE = W * E
    n_tiles = b_core // 128
    assert b_core % 128 == 0

    nc = bacc.Bacc(
        "TRN2",
        target_bir_lowering=False,
        debug=False,
        enable_asserts=False,
        num_devices=N_CORES,
    )

    value_d = nc.dram_tensor("value", [b_core, W, E], f32, kind="ExternalInput").ap()
    query_d = nc.dram_tensor("query", [b_core, E], f32, kind="ExternalInput").ap()
    w12_d = nc.dram_tensor("w12rep", [128, WE], f16, kind="ExternalInput").ap()
    w3_d = nc.dram_tensor("w3rep", [128, WE], f16, kind="ExternalInput").ap()
    ctx_d = nc.dram_tensor("ctx", [b_core, E], f32, kind="ExternalOutput").ap()

    value_f = value_d.rearrange("b w e -> b (w e)")

    mult = mybir.AluOpType.mult
    add = mybir.AluOpType.add
    bypass = mybir.AluOpType.bypass
    AXX = mybir.AxisListType.X
    Tanh = mybir.ActivationFunctionType.Tanh
    Exp = mybir.ActivationFunctionType.Exp

    with tile.TileContext(nc) as tc:
        with (
            tc.tile_pool(name="consts", bufs=1) as cpool,
            tc.tile_pool(name="vbuf", bufs=3) as vpool,
            tc.tile_pool(name="hbuf", bufs=2) as hpool,
            tc.tile_pool(name="qbuf", bufs=2) as qpool,
            tc.tile_pool(name="small", bufs=2) as spool,
            tc.tile_pool(name="ctxbuf", bufs=2) as opool,
        ):
            w12 = cpool.tile([128, WE], f16, tag="w12")
            nc.sync.dma_start(w12[:], w12_d)
            w3 = cpool.tile([128, WE], f16, tag="w3")
            nc.sync.dma_start(w3[:], w3_d)
            w12_3 = w12[:].rearrange("p (w e) -> p w e", w=W)
            w3_3 = w3[:].rearrange("p (w e) -> p w e", w=W)

            for i in range(n_tiles * reps):
                i = i % n_tiles
                rows = slice(i * 128, (i + 1) * 128)

                v16 = vpool.tile([128, WE], f16)
                nc.gpsimd.dma_start(v16[:], value_f[rows, :])
                v3 = v16[:].rearrange("p (w e) -> p w e", w=W)

                q16 = qpool.tile([128, E], f16)
                nc.gpsimd.dma_start(q16[:], query_d[rows, :])

                # h = v * q (q broadcast along W), then h *= W12rep
                h = hpool.tile([128, WE], f16)
                h3 = h[:].rearrange("p (w e) -> p w e", w=W)
                qb = q16[:].unsqueeze(1).broadcast_to([128, W, E])
                nc.vector.tensor_tensor(h3, v3, qb, mult)
                nc.vector.tensor_tensor(h[:], h[:], w12[:], mult)

                # s = tanh(h) in place
                nc.scalar.activation(h[:], h[:], Tanh)

                # scores[:, w] = sum_e s[:, w, :] * W3rep[:, w, :]
                scores = spool.tile([128, W], f32, tag="scores")
                for w in range(W):
                    nc.vector.scalar_tensor_tensor(
                        out=h3[:, w, :],
                        in0=h3[:, w, :],
                        scalar=1.0,
                        in1=w3_3[:, w, :],
                        op0=bypass,
                        op1=mult,
                        accum_out=scores[:, w : w + 1],
                    )

                # softmax over W (fp32), folded normalization
                m = spool.tile([128, 1], f32, tag="m")
                nc.vector.tensor_reduce(m[:], scores[:], AXX, mybir.AluOpType.max)
                negm = spool.tile([128, 1], f32, tag="negm")
                nc.scalar.mul(negm[:], m[:], -1.0)
                e32 = spool.tile([128, W], f32, tag="e32")
                nc.scalar.activation(e32[:], scores[:], Exp, bias=negm[:])
                ssum = spool.tile([128, 1], f32, tag="ssum")
                nc.vector.tensor_reduce(ssum[:], e32[:], AXX, add)
                rec = spool.tile([128, 1], f32, tag="rec")
                nc.vector.reciprocal(rec[:], ssum[:])
                a32 = spool.tile([128, W], f32, tag="a32")
                nc.vector.tensor_scalar(a32[:], e32[:], rec[:], None, mult)

                # ctx = sum_w a[:, w] * v[:, w, :]
                ctx16 = opool.tile([128, E], f16)
                nc.vector.tensor_scalar(ctx16[:], v3[:, 0, :], a32[:, 0:1], None, mult)
                for w in range(1, W):
                    nc.vector.scalar_tensor_tensor(
                        out=ctx16[:],
                        in0=v3[:, w, :],
                        scalar=a32[:, w : w + 1],
                        in1=ctx16[:],
                        op0=mult,
                        op1=add,
                    )

                nc.gpsimd.dma_start(ctx_d[rows, :], ctx16[:])

    nc.compile()
    return nc


def _get_nc(b_core: int):
    if b_core not in _CACHE:
        _CACHE[b_core] = _build(b_core)
    return _CACHE[b_core]


def _host_weights(W1, W2, W3):
    w12 = (W1.astype(np.float32)[0][None, :] * W2.astype(np.float32)).reshape(-1)
    w12rep = np.broadcast_to(w12, (128, W * E)).astype(np.float16)
    w3rep = np.broadcast_to(
        W3.astype(np.float32).reshape(-1), (128, W * E)
    ).astype(np.float16)
    return np.ascontiguousarray(w12rep), np.ascontiguousarray(w3rep)


def kernel(query, value, W1, W2, W3):
    import sys

    if "/opt/trn_rl_repo" not in sys.path:
        sys.path.insert(0, "/opt/trn_rl_repo")
    from concourse.bass_utils import run_bass_kernel_spmd

    query = np.asarray(query, dtype=np.float32)
    value = np.asarray(value, dtype=np.float32)
    w12rep, w3rep = _host_weights(np.asarray(W1), np.asarray(W2), np.asarray(W3))

    nc = _get_nc(B_CORE)
    in_maps = []
    for c in range(N_CORES):
        rows = slice(c * B_CORE, (c + 1) * B_CORE)
        in_maps.append(
            {
                "value": np.ascontiguousarray(value[rows]),
                "query": np.ascontiguousarray(query[rows]),
                "w12rep": w12rep,
                "w3rep": w3rep,
            }
        )

    res = run_bass_kernel_spmd(nc, in_maps, list(range(N_CORES)))
    out = np.concatenate([res.results[c]["ctx"] for c in range(N_CORES)], axis=0)
    return out.astype(np.float32)
